# revision 1
# baseline (speedup 1.0000x reference)
"""Multi-head attention (B=8, N=1024, D=768, 12 heads x 64) on 8 TRN2
NeuronCores, batch-parallel (one batch element per core, no collectives).

Per-core dataflow (everything transposed so no on-device transposes are
needed; x arrives host-transposed):
  - qkv projection as q^T,k^T (head-dim on partitions) and v (natural),
    fp32r matmuls at full PE rate
  - RoPE via a +-1 permutation matmul (rotate_half) + vector-engine
    elementwise combine against host-precomputed cos/sin tables
  - k^T stored zero-padded to 128 contraction rows per head: TRN2 matmuls
    with K<128 run ~2x slow, so S^T uses K=128 with the other head's q
    rows nulled by zero weights
  - S^T = k'.q'^T per head, exp on ScalarE (softmax scale folded into the
    activation's free affine), no max-subtraction (scores are O(5) here)
  - PV as out^T = [v|1]^T @ E^T -- the ones column yields the softmax
    denominators in psum row 64; normalization deferred past PV and
    pipelined per head-pair (reciprocal + broadcast-DMA + multiply)
  - out-projection from attnout^T with b_out folded in as a K=1 matmul;
    output is written transposed and untransposed on the host.
"""
import sys

sys.path.insert(0, "/opt/trn_rl_repo")

import numpy as np
import ml_dtypes

import concourse.bass as bass
import concourse.tile as tile_mod
from concourse import mybir
from concourse.bass_utils import run_bass_kernel_spmd
from concourse.vector_clock import ScopedClock

F32R = mybir.dt.float32r
F32 = mybir.dt.float32
BF16 = mybir.dt.bfloat16

B, N, D = 8, 1024, 768
H, DH = 12, 64
HP = H // 2          # head pairs (two heads share a 128-partition tile)
KC = D // 128        # contraction chunks for the projections
OC = 2 * KC          # q^T,k^T output-channel 128-tiles
RC = N // 128        # row chunks of the sequence
NK = N // 128        # key chunks
SCALE = DH ** -0.5


# --- walrus workaround: one sync-wait per instruction ---------------------
def _patched_drain_and_barrier(self, tick_clock, wait_clock):
    drain_inst = self.nc.sync.drain()
    wait_clock.add_sem_waits(
        drain_inst.ins, ScopedClock({None: tick_clock.global_clock})
    )
    si = drain_inst.ins.sync_info
    waits = list(si.on_wait or []) if si is not None else []
    if len(waits) > 1:
        drain_inst.ins.sync_info = mybir.SyncInfo(
            on_wait=waits[:1], on_update=list(si.on_update or [])
        )
        for w in waits[1:]:
            nop = self.nc.sync.nop(nofuse=True)
            nop.ins.sync_info = mybir.SyncInfo(on_wait=[w], on_update=[])
    self.nc.all_engine_barrier()
    assert self.sems is not None
    popped = self.nc._tile_sem_poison_stack.pop()
    assert popped is self._sem_poison
    self.nc.clear_and_free_semaphores(list(self.sems.allocated().values()))
    self.nc.all_engine_barrier()


tile_mod.TileContext._drain_and_barrier = _patched_drain_and_barrier


_split_counter = [0]


def split_sync_waits(nc, max_waits=1):
    """walrus rejects instructions carrying several sem waits; spill the
    excess onto engine-matched NOPs inserted directly before the offender."""
    for f in nc.m.functions:
        for bb in f.blocks:
            il = bb.instructions
            i = 0
            while i < len(il):
                inst = il[i]
                si = inst.sync_info
                waits = list(si.on_wait or []) if si is not None else []
                if len(waits) > max_waits:
                    inst.sync_info = mybir.SyncInfo(
                        on_wait=waits[:max_waits],
                        on_update=list(si.on_update or []),
                    )
                    rest = waits[max_waits:]
                    nops = []
                    for j in range(0, len(rest), max_waits):
                        _split_counter[0] += 1
                        nop = mybir.InstNoOp(
                            name=f"I-waitsplit-{_split_counter[0]}",
                            ins=[],
                            outs=[],
                            engine=inst.engine,
                        )
                        nop.sync_info = mybir.SyncInfo(
                            on_wait=rest[j : j + max_waits], on_update=[]
                        )
                        nops.append(nop)
                    for k, nop in enumerate(nops):
                        il.insert(i + k, nop)
                    i += len(nops)
                i += 1


def _bcast_rows(dram_ap, offset_elems, parts, free):
    """AP reading dram_ap[offset : offset+free] into `parts` partitions."""
    return bass.AP(
        tensor=dram_ap.tensor,
        offset=dram_ap.offset + offset_elems,
        ap=[[0, parts], [1, free]],
    )


def build_nc(with_bias=True):
    nc = bass.Bass()
    xt_d = nc.dram_tensor("xt", [D, N], BF16, kind="ExternalInput")
    wq_d = nc.dram_tensor("wq", [D, 3 * D], BF16, kind="ExternalInput")
    wo_d = nc.dram_tensor("wo", [D, D], BF16, kind="ExternalInput")
    bo_d = nc.dram_tensor("bo", [D], BF16, kind="ExternalInput")
    cos_d = nc.dram_tensor("cos2", [128, N], BF16, kind="ExternalInput")
    sin_d = nc.dram_tensor("sin2", [128, N], BF16, kind="ExternalInput")
    perm_d = nc.dram_tensor("perm", [128, 128], BF16, kind="ExternalInput")
    out_d = nc.dram_tensor("out", [D, N], F32, kind="ExternalOutput")
    import os as _os0

    _dbg = _os0.environ.get("K_DEBUG", "0") == "1"
    if _dbg:
        dbg_q = nc.dram_tensor("dbg_q", [128, KC, N], BF16, kind="ExternalOutput")
        dbg_k = nc.dram_tensor("dbg_k", [128, KC, N], BF16, kind="ExternalOutput")
        dbg_v = nc.dram_tensor(
            "dbg_v", [128, NK, H, DH + 1], BF16, kind="ExternalOutput"
        )
        dbg_au = nc.dram_tensor("dbg_au", [128, KC, N], F32, kind="ExternalOutput")
        dbg_sums = nc.dram_tensor("dbg_sums", [96, 128], F32, kind="ExternalOutput")
        dbg_attn = nc.dram_tensor("dbg_attn", [128, KC, N], BF16, kind="ExternalOutput")

    Exp = mybir.ActivationFunctionType.Exp
    Copy = mybir.ActivationFunctionType.Copy

    with tile_mod.TileContext(nc) as tc:
        with (
            tc.tile_pool(name="singles", bufs=1) as singles,
            tc.tile_pool(name="wq_pool", bufs=12) as wq_pool,
            tc.tile_pool(name="wo_pool", bufs=3) as wo_pool,
            tc.tile_pool(name="apool", bufs=4) as apool,
            tc.tile_pool(name="bpool", bufs=3) as bpool,
            tc.tile_pool(name="dpool", bufs=1, space="DRAM") as dpool,
        ):
            xt_sb = singles.tile([128, KC, N], BF16)
            wv_sb = singles.tile([128, KC, D], BF16)
            for kc in range(KC):
                nc.sync.dma_start(
                    out=xt_sb[:, kc, :], in_=xt_d[kc * 128 : (kc + 1) * 128, :]
                )
                nc.sync.dma_start(
                    out=wv_sb[:, kc, :],
                    in_=wq_d[kc * 128 : (kc + 1) * 128, 2 * D : 3 * D],
                )
            cos_sb = singles.tile([128, N], BF16)
            nc.sync.dma_start(out=cos_sb[:], in_=cos_d[:])
            sin_sb = singles.tile([128, N], BF16)
            nc.sync.dma_start(out=sin_sb[:], in_=sin_d[:])
            perm_sb = singles.tile([128, 128], BF16)
            nc.sync.dma_start(out=perm_sb[:], in_=perm_d[:])
            bo_sb = singles.tile([1, D], BF16)
            nc.sync.dma_start(
                out=bo_sb[:], in_=bo_d[:].rearrange("(o d) -> o d", o=1)
            )
            ones_sb = singles.tile([1, 512], BF16)
            nc.vector.memset(ones_sb[:], 1.0)
            wo_sb = singles.tile([128, KC, D], BF16)
            for c in range(KC):
                nc.sync.dma_start(
                    out=wo_sb[:, c, :], in_=wo_d[c * 128 : (c + 1) * 128, :]
                )

            v_sb = singles.tile([128, NK, H, DH + 1], BF16)
            nc.gpsimd.memset(v_sb[:, :, :, DH : DH + 1], 1.0)

            q_sb = singles.tile([128, KC, N], BF16)
            k_sb = singles.tile([128, KC, N], BF16)
            attnU_sb = singles.tile([128, KC, N], F32)
            attn_sb = singles.tile([128, KC, N], BF16)
            sums2_sb = singles.tile([96, 128], F32)
            recip2_sb = singles.tile([96, 128], F32)
            recip_d = dpool.tile([H * N], F32)
            recip_ap = recip_d[:]

            # ---- v projection: v[rows, 768] = x @ Wv -------------------
            with tc.tile_pool(name="ps_v", bufs=2, space="PSUM") as ps_v:
                for rc in range(RC):
                    vp = ps_v.tile([128, D], F32, tag="v")
                    for c0, w in ((0, 512), (512, 256)):
                        for kc in range(KC):
                            nc.tensor.matmul(
                                vp[:, c0 : c0 + w],
                                xt_sb[:, kc, rc * 128 : (rc + 1) * 128],
                                wv_sb[:, kc, c0 : c0 + w],
                                start=(kc == 0),
                                stop=(kc == KC - 1),
                            )
                    # strided copy into the [v | ones] per-head layout
                    nc.scalar.activation(
                        out=v_sb[:, rc, :, 0:DH],
                        in_=vp[:].rearrange("p (h d) -> p h d", h=H),
                        func=Copy,
                        scale=1.0,
                    )

            # ---- q^T / k^T projection + RoPE (one 128-col tile) --------
            def proj_oc(ps_qk, oc):
                col0 = oc * 128 if oc < KC else D + (oc - KC) * 128
                qkp = ps_qk.tile([128, N], F32, tag="qk", name=f"qkp{oc}")
                wts = []
                for kc in range(KC):
                    wt = wq_pool.tile([128, 128], BF16, tag="wq", name=f"wt{oc}_{kc}")
                    nc.sync.dma_start(
                        out=wt[:],
                        in_=wq_d[kc * 128 : (kc + 1) * 128, col0 : col0 + 128],
                    )
                    wts.append(wt)
                for qc in range(2):
                    for kc in range(KC):
                        nc.tensor.matmul(
                            qkp[:, qc * 512 : (qc + 1) * 512],
                            wts[kc][:],
                            xt_sb[:, kc, qc * 512 : (qc + 1) * 512],
                            start=(kc == 0),
                            stop=(kc == KC - 1),
                        )
                q0 = apool.tile([128, N], BF16, tag="q0", name=f"q0_{oc}")
                nc.vector.tensor_copy(q0[:], qkp[:])
                rotp = ps_qk.tile([128, N], F32, tag="qk", name=f"rotp{oc}")
                for qc in range(2):
                    nc.tensor.matmul(
                        rotp[:, qc * 512 : (qc + 1) * 512],
                        perm_sb[:],
                        q0[:, qc * 512 : (qc + 1) * 512],
                        start=True,
                        stop=True,
                    )
                t1 = apool.tile([128, N], BF16, tag="t1", name=f"t1_{oc}")
                nc.vector.tensor_mul(t1[:], rotp[:], sin_sb[:])
                t2 = apool.tile([128, N], BF16, tag="t2", name=f"t2_{oc}")
                nc.vector.tensor_mul(t2[:], q0[:], cos_sb[:])
                dst = q_sb if oc < KC else k_sb
                nc.vector.tensor_add(dst[:, oc % KC, :], t1[:], t2[:])

            # ---- attention: head pairs, row-packed K=64 S^T matmuls,
            # kc-paired psum tiles for wide exp, query-split for psum room
            def attn_pair(ps_att, qc, hp):
                # st tile per kc holds BOTH heads' scores side by side:
                # [128 keys, (head_a 512q | head_b 512q)] -> one wide exp
                pvs = []
                for a in range(2):
                    pv = ps_att.tile(
                        [65, 512], F32, tag=f"pv{a}", bufs=1, name=f"pv{a}_{qc}_{hp}"
                    )
                    pvs.append(pv)
                for kc in range(NK):
                    st = ps_att.tile(
                        [128, N], F32, tag="st", bufs=2, name=f"st_{qc}_{hp}_{kc}"
                    )
                    for a in range(2):
                        po = 64 * a
                        nc.tensor.matmul(
                            st[:, a * 512 : (a + 1) * 512],
                            k_sb[po : po + 64, hp, kc * 128 : (kc + 1) * 128],
                            q_sb[po : po + 64, hp, qc * 512 : (qc + 1) * 512],
                            start=True,
                            stop=True,
                        )
                    e = apool.tile([128, N], BF16, tag="e", name=f"e_{qc}_{hp}_{kc}")
                    nc.scalar.activation(out=e[:], in_=st[:], func=Exp, scale=SCALE)
                    for a in range(2):
                        nc.tensor.matmul(
                            pvs[a][:],
                            v_sb[:, kc, 2 * hp + a, :],
                            e[:, a * 512 : (a + 1) * 512],
                            start=(kc == 0),
                            stop=(kc == NK - 1),
                        )
                for a in range(2):
                    h = 2 * hp + a
                    po = 64 * a
                    pvt = bpool.tile(
                        [65, 512], F32, tag="pvt", name=f"pvt{qc}_{h}"
                    )
                    nc.vector.tensor_copy(pvt[:], pvs[a][:])
                    nc.sync.dma_start(
                        out=attnU_sb[po : po + 64, hp, qc * 512 : (qc + 1) * 512],
                        in_=pvt[0:64, :],
                    )
                    nc.sync.dma_start(
                        out=sums2_sb[h * 8 + qc * 4 : h * 8 + qc * 4 + 4, :],
                        in_=pvt[64:65, :],
                    )

            def normalize_quad(q4):
                nc.vector.reciprocal(
                    recip2_sb[q4 * 32 : (q4 + 1) * 32, :],
                    sums2_sb[q4 * 32 : (q4 + 1) * 32, :],
                )
                nc.sync.dma_start(
                    out=recip_d[:].rearrange("(p r) -> p r", p=96)[
                        q4 * 32 : (q4 + 1) * 32, :
                    ],
                    in_=recip2_sb[q4 * 32 : (q4 + 1) * 32, :],
                )
                for hp2 in (2 * q4, 2 * q4 + 1):
                    rb = bpool.tile([128, N], F32, tag="rb", name=f"rb{hp2}")
                    nc.sync.dma_start(
                        out=rb[0:64, :],
                        in_=_bcast_rows(recip_ap, (2 * hp2) * N, 64, N),
                    )
                    nc.sync.dma_start(
                        out=rb[64:128, :],
                        in_=_bcast_rows(recip_ap, (2 * hp2 + 1) * N, 64, N),
                    )
                    nc.vector.tensor_mul(
                        attn_sb[:, hp2, :], attnU_sb[:, hp2, :], rb[:]
                    )

            import os as _os

            _mode = _os.environ.get("K_MODE", "inter")
            if _mode == "v3a":
                # proj fully upfront in its own psum pool, then attention
                with tc.tile_pool(name="ps_qk", bufs=1, space="PSUM") as ps_qk:
                    for oc in range(KC):
                        proj_oc(ps_qk, oc)
                        proj_oc(ps_qk, KC + oc)
                with tc.tile_pool(name="ps_att", bufs=1, space="PSUM") as ps_att:
                    for hp in range(HP):
                        attn_pair(ps_att, 0, hp)
                    for hp in range(HP):
                        attn_pair(ps_att, 1, hp)
                        if hp % 2 == 1:
                            normalize_quad(hp // 2)
            else:
                with (
                    tc.tile_pool(name="ps_qk", bufs=1, space="PSUM") as ps_qk,
                    tc.tile_pool(name="ps_att", bufs=1, space="PSUM") as ps_att,
                ):
                    proj_oc(ps_qk, 0)
                    proj_oc(ps_qk, KC)
                    for hp in range(HP):
                        attn_pair(ps_att, 0, hp)
                        if hp + 1 < HP:
                            proj_oc(ps_qk, hp + 1)
                            proj_oc(ps_qk, KC + hp + 1)
                    for hp in range(HP):
                        attn_pair(ps_att, 1, hp)
                        if hp % 2 == 1:
                            normalize_quad(hp // 2)

            if _dbg:
                nc.sync.dma_start(out=dbg_q[:], in_=q_sb[:])
                nc.sync.dma_start(out=dbg_k[:], in_=k_sb[:])
                nc.sync.dma_start(out=dbg_v[:], in_=v_sb[:])
                nc.sync.dma_start(out=dbg_au[:], in_=attnU_sb[:])
                nc.sync.dma_start(out=dbg_sums[:], in_=sums2_sb[:])
                nc.sync.dma_start(out=dbg_attn[:], in_=attn_sb[:])

            # ---- out-projection ----------------------------------------
            with tc.tile_pool(name="ps_fin", bufs=3, space="PSUM") as ps_fin:
                for oc in range(KC):
                    fps = [
                        ps_fin.tile([128, 512], F32, tag="fin", name=f"fin{oc}_{i}")
                        for i in range(2)
                    ]
                    for c in range(KC):
                        for qc in range(2):
                            nc.tensor.matmul(
                                fps[qc][:],
                                wo_sb[:, c, oc * 128 : (oc + 1) * 128],
                                attn_sb[:, c, qc * 512 : (qc + 1) * 512],
                                start=(c == 0),
                                stop=(not with_bias and c == KC - 1),
                            )
                    for qc in range(2):
                        if with_bias:
                            nc.tensor.matmul(
                                fps[qc][:],
                                bo_sb[0:1, oc * 128 : (oc + 1) * 128],
                                ones_sb[:],
                                start=False,
                                stop=True,
                            )
                        fsb = bpool.tile([128, 512], F32, tag="fsb")
                        nc.scalar.activation(
                            out=fsb[:], in_=fps[qc][:], func=Copy, scale=1.0
                        )
                        nc.sync.dma_start(
                            out=out_d[
                                oc * 128 : (oc + 1) * 128, qc * 512 : (qc + 1) * 512
                            ],
                            in_=fsb[:],
                        )

    split_sync_waits(nc, max_waits=1)
    return nc


def _host_prep(x, w_qkv, w_out, b_out):
    bf = ml_dtypes.bfloat16
    inv_freq = 1.0 / (10000.0 ** (np.arange(0, DH, 2, dtype=np.float32) / DH))
    t = np.arange(N, dtype=np.float32)
    freqs = np.outer(t, inv_freq)
    emb = np.concatenate([freqs, freqs], axis=1)        # [N, DH]
    cos2 = np.tile(np.cos(emb).T.astype(np.float32), (2, 1)).astype(bf)
    sin2 = np.tile(np.sin(emb).T.astype(np.float32), (2, 1)).astype(bf)

    perm = np.zeros((128, 128), np.float32)
    for blk in range(2):
        o = blk * 64
        for m in range(32):
            perm[o + m + 32, o + m] = -1.0
        for m in range(32, 64):
            perm[o + m - 32, o + m] = 1.0
    perm = perm.astype(bf)

    xt = np.ascontiguousarray(x.transpose(0, 2, 1)).astype(bf)
    shared = {
        "wq": np.ascontiguousarray(w_qkv).astype(bf),
        "wo": np.ascontiguousarray(w_out).astype(bf),
        "bo": np.ascontiguousarray(b_out).astype(bf),
        "cos2": np.ascontiguousarray(cos2),
        "sin2": np.ascontiguousarray(sin2),
        "perm": np.ascontiguousarray(perm),
    }
    return [dict(shared, xt=np.ascontiguousarray(xt[i])) for i in range(B)]


_NC_CACHE = {}
LAST_EXEC_NS = [None]


def _run(in_maps, trace=False, with_bias=True):
    if with_bias not in _NC_CACHE:
        _NC_CACHE[with_bias] = build_nc(with_bias=with_bias)
    res = run_bass_kernel_spmd(
        _NC_CACHE[with_bias], in_maps, list(range(B)), trace=trace
    )
    LAST_EXEC_NS[0] = res.exec_time_ns
    out_t = np.stack([np.asarray(res.results[i]["out"]) for i in range(B)])
    return np.ascontiguousarray(out_t.transpose(0, 2, 1)).astype(np.float32)


def kernel(x, w_qkv, w_out, b_out, _trace=False):
    b_out = np.asarray(b_out, dtype=np.float32)
    in_maps = _host_prep(
        np.asarray(x, dtype=np.float32),
        np.asarray(w_qkv, dtype=np.float32),
        np.asarray(w_out, dtype=np.float32),
        b_out,
    )
    return _run(in_maps, trace=_trace, with_bias=bool(np.any(b_out)))



# revision 19
# speedup vs baseline: 1.3476x; 1.3476x over previous
"""Multi-head attention (B=8, N=1024, D=768, 12 heads x 64) on 8 TRN2
NeuronCores, batch-parallel (one batch element per core, no collectives).

Per-core dataflow (everything transposed so no on-device transposes are
needed; x arrives host-transposed):
  - warmup matmuls on zero tiles during the input DMA keep the PE HAM
    clock-gate at 8/8 so real matmuls start at 2.4 GHz
  - qkv projection as q^T,k^T (head-dim on partitions) and v (natural)
  - q/k head dims are host-interleaved in pairs (d, d+32) -> (2d, 2d+1),
    so RoPE's rotate_half is a within-quadrant partition pair-swap:
    one DVE stream_shuffle + elementwise combine against host-precomputed
    cos / sign-folded-sin tables.  No PE rotate matmul, no PE stall on
    the PSUM->SBUF cast chain.
  - S^T = k'.q'^T per head pair, both heads' K=64 matmuls row-tiled into
    one PE pass (concurrent via tile_position row groups); exp on ScalarE
    with the softmax scale folded into the activation's free affine
    (no max-subtraction; scores are O(5) here)
  - PV as out^T = [v|1]^T @ E^T -- the ones column yields the softmax
    denominators in psum row 64; normalization deferred past PV:
    reciprocal on DVE, broadcast across partitions via GpSimd
    partition_broadcast (no DRAM round-trip), multiply on DVE
  - out-projection per 128-col tile; the qc=0 half is interleaved into
    the qc=1 attention loop so only the last few matmuls trail the end.
"""
import sys

sys.path.insert(0, "/opt/trn_rl_repo")

import numpy as np
import ml_dtypes

import concourse.bass as bass
import concourse.tile as tile_mod
from concourse import mybir
from concourse.bass_utils import run_bass_kernel_spmd
from concourse.vector_clock import ScopedClock

F32 = mybir.dt.float32
BF16 = mybir.dt.bfloat16

B, N, D = 8, 1024, 768
H, DH = 12, 64
HP = H // 2          # head pairs (two heads share a 128-partition tile)
KC = D // 128        # contraction chunks for the projections
RC = N // 128        # row chunks of the sequence
NK = N // 128        # key chunks
SCALE = DH ** -0.5
N_WARM = 18          # HAM warmup matmuls during the input DMA


# --- walrus workaround: one sync-wait per instruction ---------------------
def _patched_drain_and_barrier(self, tick_clock, wait_clock):
    drain_inst = self.nc.sync.drain()
    wait_clock.add_sem_waits(
        drain_inst.ins, ScopedClock({None: tick_clock.global_clock})
    )
    si = drain_inst.ins.sync_info
    waits = list(si.on_wait or []) if si is not None else []
    if len(waits) > 1:
        drain_inst.ins.sync_info = mybir.SyncInfo(
            on_wait=waits[:1], on_update=list(si.on_update or [])
        )
        for w in waits[1:]:
            nop = self.nc.sync.nop(nofuse=True)
            nop.ins.sync_info = mybir.SyncInfo(on_wait=[w], on_update=[])
    self.nc.all_engine_barrier()
    assert self.sems is not None
    popped = self.nc._tile_sem_poison_stack.pop()
    assert popped is self._sem_poison
    self.nc.clear_and_free_semaphores(list(self.sems.allocated().values()))
    self.nc.all_engine_barrier()


tile_mod.TileContext._drain_and_barrier = _patched_drain_and_barrier


_split_counter = [0]


def split_sync_waits(nc, max_waits=1):
    """walrus rejects instructions carrying several sem waits; spill the
    excess onto engine-matched NOPs inserted directly before the offender."""
    for f in nc.m.functions:
        for bb in f.blocks:
            il = bb.instructions
            i = 0
            while i < len(il):
                inst = il[i]
                si = inst.sync_info
                waits = list(si.on_wait or []) if si is not None else []
                if len(waits) > max_waits:
                    inst.sync_info = mybir.SyncInfo(
                        on_wait=waits[:max_waits],
                        on_update=list(si.on_update or []),
                    )
                    rest = waits[max_waits:]
                    nops = []
                    for j in range(0, len(rest), max_waits):
                        _split_counter[0] += 1
                        nop = mybir.InstNoOp(
                            name=f"I-waitsplit-{_split_counter[0]}",
                            ins=[],
                            outs=[],
                            engine=inst.engine,
                        )
                        nop.sync_info = mybir.SyncInfo(
                            on_wait=rest[j : j + max_waits], on_update=[]
                        )
                        nops.append(nop)
                    for k, nop in enumerate(nops):
                        il.insert(i + k, nop)
                    i += len(nops)
                i += 1


def _bcast_rows(dram_ap, offset_elems, parts, free):
    """AP reading dram_ap[offset : offset+free] into `parts` partitions."""
    return bass.AP(
        tensor=dram_ap.tensor,
        offset=dram_ap.offset + offset_elems,
        ap=[[0, parts], [1, free]],
    )


def build_nc(with_bias=True):
    nc = bass.Bass()
    xt_d = nc.dram_tensor("xt", [D, N], BF16, kind="ExternalInput")
    wq_d = nc.dram_tensor("wq", [D, 3 * D], BF16, kind="ExternalInput")
    wo_d = nc.dram_tensor("wo", [D, D], BF16, kind="ExternalInput")
    bo_d = nc.dram_tensor("bo", [D], BF16, kind="ExternalInput")
    cos_d = nc.dram_tensor("cos2", [128, N], BF16, kind="ExternalInput")
    sin_d = nc.dram_tensor("sin2", [128, N], BF16, kind="ExternalInput")
    out_d = nc.dram_tensor("out", [D, N], BF16, kind="ExternalOutput")

    Exp = mybir.ActivationFunctionType.Exp
    Copy = mybir.ActivationFunctionType.Copy
    SWAP_MASK = [i ^ 1 for i in range(32)]

    with tile_mod.TileContext(nc) as tc:
        with (
            tc.tile_pool(name="singles", bufs=1) as singles,
            tc.tile_pool(name="wq_pool", bufs=12) as wq_pool,
            tc.tile_pool(name="apool", bufs=2) as apool,
            tc.tile_pool(name="bpool", bufs=3) as bpool,
            tc.tile_pool(name="dpool", bufs=1, space="DRAM") as dpool,
        ):
            # per-(qc, head) softmax reciprocal rows, bounced via DRAM for
            # the partition-broadcast read-back
            recip_d = dpool.tile([2 * H * 512], BF16)
            recip_ap = recip_d[:]
            warm_w = singles.tile([128, 128], BF16)
            nc.vector.memset(warm_w[:], 0.0)
            warm_x = singles.tile([128, 512], BF16)
            nc.gpsimd.memset(warm_x[:], 0.0)

            xt_sb = singles.tile([128, KC, N], BF16)
            wv_sb = singles.tile([128, KC, D], BF16)
            for kc in range(KC):
                nc.sync.dma_start(
                    out=xt_sb[:, kc, :], in_=xt_d[kc * 128 : (kc + 1) * 128, :]
                )
                nc.sync.dma_start(
                    out=wv_sb[:, kc, :],
                    in_=wq_d[kc * 128 : (kc + 1) * 128, 2 * D : 3 * D],
                )
            cos_sb = singles.tile([128, N], BF16)
            nc.sync.dma_start(out=cos_sb[:], in_=cos_d[:])
            sin_sb = singles.tile([128, N], BF16)
            nc.sync.dma_start(out=sin_sb[:], in_=sin_d[:])
            bo_sb = singles.tile([1, D], BF16)
            nc.sync.dma_start(
                out=bo_sb[:], in_=bo_d[:].rearrange("(o d) -> o d", o=1)
            )
            ones_sb = singles.tile([1, 512], BF16)
            nc.vector.memset(ones_sb[:], 1.0)
            wo_sb = singles.tile([128, KC, D], BF16)
            for c in range(KC):
                nc.sync.dma_start(
                    out=wo_sb[:, c, :], in_=wo_d[c * 128 : (c + 1) * 128, :]
                )

            v_sb = singles.tile([128, NK, H, DH + 1], BF16)
            nc.gpsimd.memset(v_sb[:, :, :, DH : DH + 1], 1.0)

            q_sb = singles.tile([128, KC, N], BF16)
            k_sb = singles.tile([128, KC, N], BF16)
            attnU_sb = singles.tile([128, KC, N], BF16)
            attn_sb = singles.tile([128, KC, N], BF16)


            # ---- v projection: v[rows, 768] = x @ Wv -------------------
            with tc.tile_pool(name="ps_v", bufs=2, space="PSUM") as ps_v:
                for i in range(N_WARM):
                    wp = ps_v.tile([128, 512], F32, tag="warm", name=f"warm{i}")
                    nc.tensor.matmul(
                        wp[:], warm_w[:], warm_x[:], start=True, stop=True
                    )
                for rc in range(RC):
                    vp = ps_v.tile([128, D], F32, tag="v")
                    for c0, w in ((0, 512), (512, 256)):
                        for kc in range(KC):
                            nc.tensor.matmul(
                                vp[:, c0 : c0 + w],
                                xt_sb[:, kc, rc * 128 : (rc + 1) * 128],
                                wv_sb[:, kc, c0 : c0 + w],
                                start=(kc == 0),
                                stop=(kc == KC - 1),
                            )
                    # strided copy into the [v | ones] per-head layout
                    nc.scalar.activation(
                        out=v_sb[:, rc, :, 0:DH],
                        in_=vp[:].rearrange("p (h d) -> p h d", h=H),
                        func=Copy,
                        scale=1.0,
                    )

            # ---- q^T / k^T projection + RoPE (one 128-col tile) --------
            # PE does only the 12 qkp matmuls; cast / pair-swap / rope
            # combine all run downstream on DVE.
            def proj_qk(ps_proj, oc):
                col0 = oc * 128 if oc < KC else D + (oc - KC) * 128
                wts = []
                for kc in range(KC):
                    wt = wq_pool.tile([128, 128], BF16, tag="wq", name=f"wt{oc}_{kc}")
                    nc.sync.dma_start(
                        out=wt[:],
                        in_=wq_d[kc * 128 : (kc + 1) * 128, col0 : col0 + 128],
                    )
                    wts.append(wt)
                q0 = apool.tile([128, N], BF16, tag="q0", name=f"q0_{oc}")
                for qc2 in range(2):
                    qkp = ps_proj.tile(
                        [128, 512], F32, tag="pq", name=f"qkp{oc}_{qc2}"
                    )
                    for kc in range(KC):
                        nc.tensor.matmul(
                            qkp[:],
                            wts[kc][:],
                            xt_sb[:, kc, qc2 * 512 : (qc2 + 1) * 512],
                            start=(kc == 0),
                            stop=(kc == KC - 1),
                        )
                    nc.vector.tensor_copy(
                        q0[:, qc2 * 512 : (qc2 + 1) * 512], qkp[:]
                    )
                q0s = apool.tile([128, N], BF16, tag="q0s", name=f"q0s_{oc}")
                nc.vector.stream_shuffle(q0s[:], q0[:], SWAP_MASK)
                t1 = apool.tile([128, N], BF16, tag="t1", name=f"t1_{oc}")
                nc.vector.tensor_mul(t1[:], q0s[:], sin_sb[:])
                t2 = apool.tile([128, N], BF16, tag="t2", name=f"t2_{oc}")
                nc.vector.tensor_mul(t2[:], q0[:], cos_sb[:])
                dst = q_sb if oc < KC else k_sb
                nc.vector.tensor_add(dst[:, oc % KC, :], t1[:], t2[:])

            # ---- attention: head pairs, row-tiled K=64 S^T matmuls -----
            def attn_pair(ps_att, qc, hp):
                pvs = []
                for a in range(2):
                    pv = ps_att.tile(
                        [65, 512], F32, tag=f"pv{a}", bufs=1, name=f"pv{a}_{qc}_{hp}"
                    )
                    pvs.append(pv)
                for kc in range(NK):
                    st = ps_att.tile(
                        [128, N], F32, tag="st", bufs=2, name=f"st_{qc}_{hp}_{kc}"
                    )
                    for a in range(2):
                        po = 64 * a
                        nc.tensor.matmul(
                            st[:, a * 512 : (a + 1) * 512],
                            k_sb[po : po + 64, hp, kc * 128 : (kc + 1) * 128],
                            q_sb[po : po + 64, hp, qc * 512 : (qc + 1) * 512],
                            start=True,
                            stop=True,
                        )
                    e = apool.tile([128, N], BF16, tag="e", name=f"e_{qc}_{hp}_{kc}")
                    nc.scalar.activation(out=e[:], in_=st[:], func=Exp, scale=SCALE)
                    for a in range(2):
                        nc.tensor.matmul(
                            pvs[a][:],
                            v_sb[:, kc, 2 * hp + a, :],
                            e[:, a * 512 : (a + 1) * 512],
                            start=(kc == 0),
                            stop=(kc == NK - 1),
                        )
                sums_t = bpool.tile([8, 128], BF16, tag="sums", name=f"sums{qc}_{hp}")
                for a in range(2):
                    po = 64 * a
                    pvt = bpool.tile(
                        [65, 512], BF16, tag="pvt", name=f"pvt{qc}_{2*hp+a}"
                    )
                    nc.vector.tensor_copy(pvt[:], pvs[a][:])
                    nc.sync.dma_start(
                        out=attnU_sb[po : po + 64, hp, qc * 512 : (qc + 1) * 512],
                        in_=pvt[0:64, :],
                    )
                    nc.sync.dma_start(
                        out=sums_t[a * 4 : a * 4 + 4, :], in_=pvt[64:65, :]
                    )
                return sums_t

            # ---- softmax normalization for one (head pair, qc) --------
            def norm(hp, qc, sums_t):
                rcp = bpool.tile([8, 128], BF16, tag="rcp", name=f"rcp{hp}_{qc}")
                with nc.allow_low_precision(
                    reason="bf16 softmax denominators; rel-err budget is 2e-2"
                ):
                    nc.vector.reciprocal(rcp[:], sums_t[:])
                o0 = (qc * H + 2 * hp) * 512
                nc.sync.dma_start(
                    out=recip_d[o0 : o0 + 1024].rearrange("(p f) -> p f", p=8),
                    in_=rcp[:],
                )
                rb = bpool.tile([128, 512], BF16, tag="rb", name=f"rb{hp}_{qc}")
                nc.sync.dma_start(
                    out=rb[0:64, :], in_=_bcast_rows(recip_ap, o0, 64, 512)
                )
                nc.sync.dma_start(
                    out=rb[64:128, :], in_=_bcast_rows(recip_ap, o0 + 512, 64, 512)
                )
                nc.vector.tensor_mul(
                    attn_sb[:, hp, qc * 512 : (qc + 1) * 512],
                    attnU_sb[:, hp, qc * 512 : (qc + 1) * 512],
                    rb[:],
                )

            # ---- out-projection for one (128-col tile, qc) -------------
            def outproj(ps_fin, oc, qc):
                fp = ps_fin.tile([128, 512], F32, tag="fin", name=f"fin{oc}_{qc}")
                for c in range(KC):
                    nc.tensor.matmul(
                        fp[:],
                        wo_sb[:, c, oc * 128 : (oc + 1) * 128],
                        attn_sb[:, c, qc * 512 : (qc + 1) * 512],
                        start=(c == 0),
                        stop=(not with_bias and c == KC - 1),
                    )
                if with_bias:
                    nc.tensor.matmul(
                        fp[:],
                        bo_sb[0:1, oc * 128 : (oc + 1) * 128],
                        ones_sb[:],
                        start=False,
                        stop=True,
                    )
                fsb = bpool.tile([128, 512], BF16, tag="fsb", name=f"fsb{oc}_{qc}")
                nc.vector.tensor_copy(fsb[:], fp[:])
                nc.sync.dma_start(
                    out=out_d[oc * 128 : (oc + 1) * 128, qc * 512 : (qc + 1) * 512],
                    in_=fsb[:],
                )

            with tc.tile_pool(name="ps_att", bufs=1, space="PSUM") as ps_att:
                with tc.tile_pool(name="ps_proj", bufs=2, space="PSUM") as ps_proj:
                    # 2-deep proj pipeline so hp=0's DVE rope chain hides
                    # under hp=1's qkp matmuls
                    proj_qk(ps_proj, 0)
                    proj_qk(ps_proj, KC)
                    proj_qk(ps_proj, 1)
                    proj_qk(ps_proj, KC + 1)
                    for hp in range(HP):
                        s_t = attn_pair(ps_att, 0, hp)
                        norm(hp, 0, s_t)
                        if hp + 2 < HP:
                            proj_qk(ps_proj, hp + 2)
                            proj_qk(ps_proj, KC + hp + 2)
                with tc.tile_pool(name="ps_fin", bufs=2, space="PSUM") as ps_fin:
                    for hp in range(HP):
                        s_t = attn_pair(ps_att, 1, hp)
                        norm(hp, 1, s_t)
                        outproj(ps_fin, hp, 0)
                    for oc in range(KC):
                        outproj(ps_fin, oc, 1)

    split_sync_waits(nc, max_waits=1)
    return nc


def _host_prep(x, w_qkv, w_out, b_out):
    bf = ml_dtypes.bfloat16
    inv_freq = 1.0 / (10000.0 ** (np.arange(0, DH, 2, dtype=np.float32) / DH))
    t = np.arange(N, dtype=np.float32)
    freqs = np.outer(t, inv_freq)
    emb = np.concatenate([freqs, freqs], axis=1)        # [N, DH]
    cos_t = np.cos(emb).T.astype(np.float32)            # [DH, N]
    sin_t = np.sin(emb).T.astype(np.float32)

    # interleave head dims in pairs (d, d+32) -> rows (2d, 2d+1) so
    # rotate_half becomes a partition pair-swap; fold rotate's sign into
    # the sin table (row 2d carries -sin)
    perm64 = np.ravel(
        np.stack([np.arange(32), np.arange(32) + 32], axis=1)
    )                                                   # [0,32,1,33,...]
    signs = np.tile(np.array([-1.0, 1.0], np.float32), 32)[:, None]
    cos_p = cos_t[perm64]
    sin_p = sin_t[perm64] * signs
    cos2 = np.tile(cos_p, (2, 1)).astype(bf)
    sin2 = np.tile(sin_p, (2, 1)).astype(bf)

    inner = H * DH
    qk_perm = np.concatenate([h * DH + perm64 for h in range(H)])
    wq = np.asarray(w_qkv, dtype=np.float32).copy()
    wq[:, 0:inner] = wq[:, 0:inner][:, qk_perm]
    wq[:, inner : 2 * inner] = wq[:, inner : 2 * inner][:, qk_perm]

    xt = np.ascontiguousarray(x.transpose(0, 2, 1)).astype(bf)
    shared = {
        "wq": np.ascontiguousarray(wq).astype(bf),
        "wo": np.ascontiguousarray(w_out).astype(bf),
        "bo": np.ascontiguousarray(b_out).astype(bf),
        "cos2": np.ascontiguousarray(cos2),
        "sin2": np.ascontiguousarray(sin2),
    }
    return [dict(shared, xt=np.ascontiguousarray(xt[i])) for i in range(B)]


_NC_CACHE = {}
LAST_EXEC_NS = [None]


def _run(in_maps, trace=False, with_bias=True):
    if with_bias not in _NC_CACHE:
        _NC_CACHE[with_bias] = build_nc(with_bias=with_bias)
    res = run_bass_kernel_spmd(
        _NC_CACHE[with_bias], in_maps, list(range(B)), trace=trace
    )
    LAST_EXEC_NS[0] = res.exec_time_ns
    out_t = np.stack(
        [np.asarray(res.results[i]["out"]).astype(np.float32) for i in range(B)]
    )
    return np.ascontiguousarray(out_t.transpose(0, 2, 1))


def kernel(x, w_qkv, w_out, b_out, _trace=False):
    b_out = np.asarray(b_out, dtype=np.float32)
    in_maps = _host_prep(
        np.asarray(x, dtype=np.float32),
        np.asarray(w_qkv, dtype=np.float32),
        np.asarray(w_out, dtype=np.float32),
        b_out,
    )
    return _run(in_maps, trace=_trace, with_bias=bool(np.any(b_out)))


# revision 20
# speedup vs baseline: 1.3783x; 1.0228x over previous
"""Multi-head attention (B=8, N=1024, D=768, 12 heads x 64) on 8 TRN2
NeuronCores, batch-parallel (one batch element per core, no collectives).

Per-core dataflow (everything transposed so no on-device transposes are
needed; x arrives host-transposed):
  - warmup matmuls on zero tiles during the input DMA keep the PE HAM
    clock-gate at 8/8 so real matmuls start at 2.4 GHz
  - qkv projection as q^T,k^T (head-dim on partitions) and v (natural)
  - q/k head dims are host-interleaved in pairs (d, d+32) -> (2d, 2d+1),
    so RoPE's rotate_half is a within-quadrant partition pair-swap:
    one DVE stream_shuffle + elementwise combine against host-precomputed
    cos / sign-folded-sin tables.  No PE rotate matmul.
  - S^T = k'.q'^T per head pair, both heads' K=64 matmuls row-tiled into
    one PE pass (concurrent via tile_position row groups); exp on ScalarE
    with the softmax scale folded into the activation's free affine
  - PV as out^T = [v|1]^T @ E^T -- the ones column yields the softmax
    denominators in psum row 64; normalization deferred past PV:
    reciprocal on DVE, partition-broadcast via a DRAM bounce for interior
    pairs (latency hidden) and via indicator-weight matmuls into PSUM for
    the final pair (keeps the critical tail on-chip)
  - projection / out-projection matmul chunks are interleaved into the
    attention kc loop so the PE has filler work while each kc's exp cooks
  - out-projection's qc=0 half runs during the qc=1 attention loop; only
    the last few matmuls trail the end.
"""
import sys

sys.path.insert(0, "/opt/trn_rl_repo")

from collections import deque

import numpy as np
import ml_dtypes

import concourse.bass as bass
import concourse.tile as tile_mod
from concourse import mybir
from concourse.bass_utils import run_bass_kernel_spmd
from concourse.vector_clock import ScopedClock

F32 = mybir.dt.float32
BF16 = mybir.dt.bfloat16

B, N, D = 8, 1024, 768
H, DH = 12, 64
HP = H // 2          # head pairs (two heads share a 128-partition tile)
KC = D // 128        # contraction chunks for the projections
RC = N // 128        # row chunks of the sequence
NK = N // 128        # key chunks
SCALE = DH ** -0.5
N_WARM = 10          # HAM warmup matmuls during the input DMA


# --- walrus workaround: one sync-wait per instruction ---------------------
def _patched_drain_and_barrier(self, tick_clock, wait_clock):
    drain_inst = self.nc.sync.drain()
    wait_clock.add_sem_waits(
        drain_inst.ins, ScopedClock({None: tick_clock.global_clock})
    )
    si = drain_inst.ins.sync_info
    waits = list(si.on_wait or []) if si is not None else []
    if len(waits) > 1:
        drain_inst.ins.sync_info = mybir.SyncInfo(
            on_wait=waits[:1], on_update=list(si.on_update or [])
        )
        for w in waits[1:]:
            nop = self.nc.sync.nop(nofuse=True)
            nop.ins.sync_info = mybir.SyncInfo(on_wait=[w], on_update=[])
    self.nc.all_engine_barrier()
    assert self.sems is not None
    popped = self.nc._tile_sem_poison_stack.pop()
    assert popped is self._sem_poison
    self.nc.clear_and_free_semaphores(list(self.sems.allocated().values()))
    self.nc.all_engine_barrier()


tile_mod.TileContext._drain_and_barrier = _patched_drain_and_barrier


_split_counter = [0]


def split_sync_waits(nc, max_waits=1):
    """walrus rejects instructions carrying several sem waits; spill the
    excess onto engine-matched NOPs inserted directly before the offender."""
    for f in nc.m.functions:
        for bb in f.blocks:
            il = bb.instructions
            i = 0
            while i < len(il):
                inst = il[i]
                si = inst.sync_info
                waits = list(si.on_wait or []) if si is not None else []
                if len(waits) > max_waits:
                    inst.sync_info = mybir.SyncInfo(
                        on_wait=waits[:max_waits],
                        on_update=list(si.on_update or []),
                    )
                    rest = waits[max_waits:]
                    nops = []
                    for j in range(0, len(rest), max_waits):
                        _split_counter[0] += 1
                        nop = mybir.InstNoOp(
                            name=f"I-waitsplit-{_split_counter[0]}",
                            ins=[],
                            outs=[],
                            engine=inst.engine,
                        )
                        nop.sync_info = mybir.SyncInfo(
                            on_wait=rest[j : j + max_waits], on_update=[]
                        )
                        nops.append(nop)
                    for k, nop in enumerate(nops):
                        il.insert(i + k, nop)
                    i += len(nops)
                i += 1


def _bcast_rows(dram_ap, offset_elems, parts, free):
    """AP reading dram_ap[offset : offset+free] into `parts` partitions."""
    return bass.AP(
        tensor=dram_ap.tensor,
        offset=dram_ap.offset + offset_elems,
        ap=[[0, parts], [1, free]],
    )


def build_nc(with_bias=True):
    nc = bass.Bass()
    xt_d = nc.dram_tensor("xt", [D, N], BF16, kind="ExternalInput")
    wq_d = nc.dram_tensor("wq", [D, 3 * D], BF16, kind="ExternalInput")
    wo_d = nc.dram_tensor("wo", [D, D], BF16, kind="ExternalInput")
    bo_d = nc.dram_tensor("bo", [D], BF16, kind="ExternalInput")
    cos_d = nc.dram_tensor("cos2", [128, N], BF16, kind="ExternalInput")
    sin_d = nc.dram_tensor("sin2", [128, N], BF16, kind="ExternalInput")
    sel_d = nc.dram_tensor("sel", [8, 512], BF16, kind="ExternalInput")
    out_d = nc.dram_tensor("out", [D, N], BF16, kind="ExternalOutput")

    Exp = mybir.ActivationFunctionType.Exp
    Copy = mybir.ActivationFunctionType.Copy
    SWAP_MASK = [i ^ 1 for i in range(32)]

    with tile_mod.TileContext(nc) as tc:
        with (
            tc.tile_pool(name="singles", bufs=1) as singles,
            tc.tile_pool(name="work", bufs=2) as work,
            tc.tile_pool(name="dpool", bufs=1, space="DRAM") as dpool,
        ):
            recip_d = dpool.tile([2 * H * 512], BF16)
            recip_ap = recip_d[:]

            warm_w = singles.tile([128, 128], BF16)
            nc.vector.memset(warm_w[:], 0.0)
            warm_x = singles.tile([128, 512], BF16)
            nc.vector.memset(warm_x[:], 0.0)

            # input loads: xt/wv interleaved per-chunk on SyncE so the
            # first v-proj matmuls can start as soon as chunk 0 lands;
            # everything else issues in parallel on ScalarE (2nd HWDGE)
            xt_sb = singles.tile([128, KC, N], BF16)
            wv_sb = singles.tile([128, KC, D], BF16)
            for kc in range(KC):
                nc.sync.dma_start(
                    out=xt_sb[:, kc, :], in_=xt_d[kc * 128 : (kc + 1) * 128, :]
                )
                nc.sync.dma_start(
                    out=wv_sb[:, kc, :],
                    in_=wq_d[kc * 128 : (kc + 1) * 128, 2 * D : 3 * D],
                )
            sel_sb = singles.tile([8, 4, 128], BF16)
            nc.scalar.dma_start(
                out=sel_sb[:], in_=sel_d[:].rearrange("p (j f) -> p j f", j=4)
            )
            cos_sb = singles.tile([128, N], BF16)
            nc.scalar.dma_start(out=cos_sb[:], in_=cos_d[:])
            sin_sb = singles.tile([128, N], BF16)
            nc.scalar.dma_start(out=sin_sb[:], in_=sin_d[:])
            bo_sb = singles.tile([1, D], BF16)
            nc.scalar.dma_start(
                out=bo_sb[:], in_=bo_d[:].rearrange("(o d) -> o d", o=1)
            )
            ones_sb = singles.tile([1, 512], BF16)
            nc.vector.memset(ones_sb[:], 1.0)
            wo_sb = singles.tile([128, KC, D], BF16)
            for c in range(KC):
                nc.scalar.dma_start(
                    out=wo_sb[:, c, :], in_=wo_d[c * 128 : (c + 1) * 128, :]
                )

            v_sb = singles.tile([128, NK, H, DH + 1], BF16)
            nc.gpsimd.memset(v_sb[:, :, :, DH : DH + 1], 1.0)

            q_sb = singles.tile([128, KC, N], BF16)
            k_sb = singles.tile([128, KC, N], BF16)
            attnU_sb = singles.tile([128, KC, N], BF16)
            attn_sb = singles.tile([128, KC, N], BF16)

            # ---- v projection: v[rows, 768] = x @ Wv -------------------
            with tc.tile_pool(name="ps_v", bufs=2, space="PSUM") as ps_v:
                for i in range(N_WARM):
                    wp = ps_v.tile([128, 512], F32, tag="warm", name=f"warm{i}")
                    nc.tensor.matmul(
                        wp[:], warm_w[:], warm_x[:], start=True, stop=True
                    )
                for rc in range(RC):
                    vp = ps_v.tile([128, D], F32, tag="v")
                    for c0, w in ((0, 512), (512, 256)):
                        for kc in range(KC):
                            nc.tensor.matmul(
                                vp[:, c0 : c0 + w],
                                xt_sb[:, kc, rc * 128 : (rc + 1) * 128],
                                wv_sb[:, kc, c0 : c0 + w],
                                start=(kc == 0),
                                stop=(kc == KC - 1),
                            )
                    # strided copy into the [v | ones] per-head layout
                    nc.scalar.activation(
                        out=v_sb[:, rc, :, 0:DH],
                        in_=vp[:].rearrange("p (h d) -> p h d", h=H),
                        func=Copy,
                        scale=1.0,
                    )

            # ---- q^T / k^T projection + RoPE, split into filler chunks -
            # PE does only the 12 qkp matmuls; cast / pair-swap / rope
            # combine all run downstream on DVE.
            def proj_chunks(ps_proj, oc):
                col0 = oc * 128 if oc < KC else D + (oc - KC) * 128
                wt = work.tile(
                    [128, KC, 128], BF16, tag="wq", bufs=4, name=f"wt{oc}"
                )
                nc.sync.dma_start(
                    out=wt[:],
                    in_=wq_d[:, col0 : col0 + 128].rearrange(
                        "(c p) m -> p c m", c=KC
                    ),
                )
                q0 = work.tile([128, N], BF16, tag="q0", bufs=2, name=f"q0_{oc}")

                def half(qc2):
                    qkp = ps_proj.tile(
                        [128, 512], F32, tag="pq", name=f"qkp{oc}_{qc2}"
                    )
                    for kc in range(KC):
                        nc.tensor.matmul(
                            qkp[:],
                            wt[:, kc, :],
                            xt_sb[:, kc, qc2 * 512 : (qc2 + 1) * 512],
                            start=(kc == 0),
                            stop=(kc == KC - 1),
                        )
                    nc.vector.tensor_copy(
                        q0[:, qc2 * 512 : (qc2 + 1) * 512], qkp[:]
                    )

                def rope_tail():
                    q0s = work.tile(
                        [128, N], BF16, tag="q0s", bufs=2, name=f"q0s_{oc}"
                    )
                    nc.vector.stream_shuffle(q0s[:], q0[:], SWAP_MASK)
                    t1 = work.tile([128, N], BF16, tag="t1", bufs=2, name=f"t1_{oc}")
                    nc.vector.tensor_mul(t1[:], q0s[:], sin_sb[:])
                    t2 = work.tile([128, N], BF16, tag="t2", bufs=2, name=f"t2_{oc}")
                    nc.vector.tensor_mul(t2[:], q0[:], cos_sb[:])
                    dst = q_sb if oc < KC else k_sb
                    nc.vector.tensor_add(dst[:, oc % KC, :], t1[:], t2[:])

                return [lambda: half(0), lambda: half(1), rope_tail]

            # ---- attention: head pairs, row-tiled K=64 S^T matmuls -----
            def attn_pair(ps_att, qc, hp, filler):
                pvs = []
                for a in range(2):
                    pv = ps_att.tile(
                        [65, 512], F32, tag=f"pv{a}", bufs=1, name=f"pv{a}_{qc}_{hp}"
                    )
                    pvs.append(pv)
                for kc in range(NK):
                    st = ps_att.tile(
                        [128, N], F32, tag="st", bufs=2, name=f"st_{qc}_{hp}_{kc}"
                    )
                    for a in range(2):
                        po = 64 * a
                        nc.tensor.matmul(
                            st[:, a * 512 : (a + 1) * 512],
                            k_sb[po : po + 64, hp, kc * 128 : (kc + 1) * 128],
                            q_sb[po : po + 64, hp, qc * 512 : (qc + 1) * 512],
                            start=True,
                            stop=True,
                        )
                    e = work.tile([128, N], BF16, tag="e", bufs=2, name=f"e_{qc}_{hp}_{kc}")
                    nc.scalar.activation(out=e[:], in_=st[:], func=Exp, scale=SCALE)
                    for a in range(2):
                        nc.tensor.matmul(
                            pvs[a][:],
                            v_sb[:, kc, 2 * hp + a, :],
                            e[:, a * 512 : (a + 1) * 512],
                            start=(kc == 0),
                            stop=(kc == NK - 1),
                        )
                    if filler:
                        filler.popleft()()
                sums_t = work.tile([8, 128], BF16, tag="sums", bufs=2, name=f"sums{qc}_{hp}")
                for a in range(2):
                    po = 64 * a
                    pvt = work.tile(
                        [65, 512], BF16, tag="pvt", bufs=3, name=f"pvt{qc}_{2*hp+a}"
                    )
                    nc.vector.tensor_copy(pvt[:], pvs[a][:])
                    nc.sync.dma_start(
                        out=attnU_sb[po : po + 64, hp, qc * 512 : (qc + 1) * 512],
                        in_=pvt[0:64, :],
                    )
                    nc.sync.dma_start(
                        out=sums_t[a * 4 : a * 4 + 4, :], in_=pvt[64:65, :]
                    )
                return sums_t

            # ---- softmax normalization for one (head pair, qc) --------
            # interior pairs broadcast the reciprocal rows via a DRAM
            # bounce (latency hidden under later work); the final pair
            # uses indicator-weight matmuls into PSUM to stay on-chip
            def norm(hp, qc, sums_t, ps_last=None):
                rcp = work.tile([8, 128], BF16, tag="rcp", bufs=2, name=f"rcp{hp}_{qc}")
                with nc.allow_low_precision(
                    reason="bf16 softmax denominators; rel-err budget is 2e-2"
                ):
                    nc.vector.reciprocal(rcp[:], sums_t[:])
                if ps_last is not None:
                    rbp = ps_last.tile([128, 512], F32, tag="fin", name=f"rbp{hp}_{qc}")
                    for j in range(4):
                        nc.tensor.matmul(
                            rbp[:, j * 128 : (j + 1) * 128],
                            sel_sb[:, j, :],
                            rcp[:],
                            start=(j == 0),
                            stop=(j == 3),
                        )
                    nc.vector.tensor_mul(
                        attn_sb[:, hp, qc * 512 : (qc + 1) * 512],
                        attnU_sb[:, hp, qc * 512 : (qc + 1) * 512],
                        rbp[:],
                    )
                    return
                o0 = (qc * H + 2 * hp) * 512
                nc.sync.dma_start(
                    out=recip_d[o0 : o0 + 1024].rearrange("(p f) -> p f", p=8),
                    in_=rcp[:],
                )
                rb = work.tile([128, 512], BF16, tag="rb", bufs=3, name=f"rb{hp}_{qc}")
                nc.sync.dma_start(
                    out=rb[0:64, :], in_=_bcast_rows(recip_ap, o0, 64, 512)
                )
                nc.sync.dma_start(
                    out=rb[64:128, :], in_=_bcast_rows(recip_ap, o0 + 512, 64, 512)
                )
                nc.vector.tensor_mul(
                    attn_sb[:, hp, qc * 512 : (qc + 1) * 512],
                    attnU_sb[:, hp, qc * 512 : (qc + 1) * 512],
                    rb[:],
                )

            # ---- out-projection for one (128-col tile, qc) -------------
            def outproj_chunks(ps_fin, oc, qc, tail_cast_act=False):
                fp = ps_fin.tile([128, 512], F32, tag="fin", name=f"fin{oc}_{qc}")

                def cpair(c0):
                    last = c0 + 2 >= KC
                    for c in (c0, c0 + 1):
                        nc.tensor.matmul(
                            fp[:],
                            wo_sb[:, c, oc * 128 : (oc + 1) * 128],
                            attn_sb[:, c, qc * 512 : (qc + 1) * 512],
                            start=(c == 0),
                            stop=(not with_bias and c == KC - 1),
                        )
                    if not last:
                        return
                    if with_bias:
                        nc.tensor.matmul(
                            fp[:],
                            bo_sb[0:1, oc * 128 : (oc + 1) * 128],
                            ones_sb[:],
                            start=False,
                            stop=True,
                        )
                    fsb = work.tile(
                        [128, 512], BF16, tag="fsb", bufs=3, name=f"fsb{oc}_{qc}"
                    )
                    if tail_cast_act:
                        nc.scalar.activation(
                            out=fsb[:], in_=fp[:], func=Copy, scale=1.0
                        )
                    else:
                        nc.vector.tensor_copy(fsb[:], fp[:])
                    eng = nc.scalar if tail_cast_act else nc.sync
                    eng.dma_start(
                        out=out_d[
                            oc * 128 : (oc + 1) * 128, qc * 512 : (qc + 1) * 512
                        ],
                        in_=fsb[:],
                    )

                return [lambda: cpair(0), lambda: cpair(2), lambda: cpair(4)]

            with tc.tile_pool(name="ps_att", bufs=1, space="PSUM") as ps_att:
                pend = deque()
                with tc.tile_pool(name="ps_proj", bufs=2, space="PSUM") as ps_proj:
                    for f in proj_chunks(ps_proj, 0) + proj_chunks(ps_proj, KC):
                        f()
                    for hp in range(HP):
                        if hp + 1 < HP:
                            pend.extend(proj_chunks(ps_proj, hp + 1))
                            pend.extend(proj_chunks(ps_proj, KC + hp + 1))
                        s_t = attn_pair(ps_att, 0, hp, pend)
                        while pend:
                            pend.popleft()()
                        norm(hp, 0, s_t)
                with tc.tile_pool(name="ps_fin", bufs=2, space="PSUM") as ps_fin:
                    for hp in range(HP):
                        pend.extend(outproj_chunks(ps_fin, hp, 0))
                        s_t = attn_pair(ps_att, 1, hp, pend)
                        while pend:
                            pend.popleft()()
                        norm(
                            hp, 1, s_t,
                            ps_last=ps_fin if hp == HP - 1 else None,
                        )
                    for oc in range(KC):
                        for f in outproj_chunks(ps_fin, oc, 1, tail_cast_act=(oc % 2 == 1)):
                            f()

    split_sync_waits(nc, max_waits=1)
    return nc


def _host_prep(x, w_qkv, w_out, b_out):
    bf = ml_dtypes.bfloat16
    inv_freq = 1.0 / (10000.0 ** (np.arange(0, DH, 2, dtype=np.float32) / DH))
    t = np.arange(N, dtype=np.float32)
    freqs = np.outer(t, inv_freq)
    emb = np.concatenate([freqs, freqs], axis=1)        # [N, DH]
    cos_t = np.cos(emb).T.astype(np.float32)            # [DH, N]
    sin_t = np.sin(emb).T.astype(np.float32)

    # interleave head dims in pairs (d, d+32) -> rows (2d, 2d+1) so
    # rotate_half becomes a partition pair-swap; fold rotate's sign into
    # the sin table (row 2d carries -sin)
    perm64 = np.ravel(
        np.stack([np.arange(32), np.arange(32) + 32], axis=1)
    )                                                   # [0,32,1,33,...]
    signs = np.tile(np.array([-1.0, 1.0], np.float32), 32)[:, None]
    cos_p = cos_t[perm64]
    sin_p = sin_t[perm64] * signs
    cos2 = np.tile(cos_p, (2, 1)).astype(bf)
    sin2 = np.tile(sin_p, (2, 1)).astype(bf)

    inner = H * DH
    qk_perm = np.concatenate([h * DH + perm64 for h in range(H)])
    wq = np.asarray(w_qkv, dtype=np.float32).copy()
    wq[:, 0:inner] = wq[:, 0:inner][:, qk_perm]
    wq[:, inner : 2 * inner] = wq[:, inner : 2 * inner][:, qk_perm]

    # indicator weights for the final-pair reciprocal broadcast:
    # sel[h*4+j, j, h*64:(h+1)*64] = 1
    sel = np.zeros((8, 4, 128), np.float32)
    for h in range(2):
        for j in range(4):
            sel[h * 4 + j, j, h * 64 : (h + 1) * 64] = 1.0

    xt = np.ascontiguousarray(x.transpose(0, 2, 1)).astype(bf)
    shared = {
        "wq": np.ascontiguousarray(wq).astype(bf),
        "wo": np.ascontiguousarray(w_out).astype(bf),
        "bo": np.ascontiguousarray(b_out).astype(bf),
        "cos2": np.ascontiguousarray(cos2),
        "sin2": np.ascontiguousarray(sin2),
        "sel": np.ascontiguousarray(sel.reshape(8, 512)).astype(bf),
    }
    return [dict(shared, xt=np.ascontiguousarray(xt[i])) for i in range(B)]


_NC_CACHE = {}
LAST_EXEC_NS = [None]


def _run(in_maps, trace=False, with_bias=True):
    if with_bias not in _NC_CACHE:
        _NC_CACHE[with_bias] = build_nc(with_bias=with_bias)
    res = run_bass_kernel_spmd(
        _NC_CACHE[with_bias], in_maps, list(range(B)), trace=trace
    )
    LAST_EXEC_NS[0] = res.exec_time_ns
    out_t = np.stack(
        [np.asarray(res.results[i]["out"]).astype(np.float32) for i in range(B)]
    )
    return np.ascontiguousarray(out_t.transpose(0, 2, 1))


def kernel(x, w_qkv, w_out, b_out, _trace=False):
    b_out = np.asarray(b_out, dtype=np.float32)
    in_maps = _host_prep(
        np.asarray(x, dtype=np.float32),
        np.asarray(w_qkv, dtype=np.float32),
        np.asarray(w_out, dtype=np.float32),
        b_out,
    )
    return _run(in_maps, trace=_trace, with_bias=bool(np.any(b_out)))


# revision 41
# speedup vs baseline: 1.4347x; 1.0409x over previous
"""Multi-head attention (B=8, N=1024, D=768, 12 heads x 64) on 8 TRN2
NeuronCores, batch-parallel (one batch element per core, no collectives).

Per-core dataflow (everything transposed so no on-device transposes are
needed; x arrives host-transposed):
  - warmup matmuls on zero tiles during the input DMA keep the PE HAM
    clock-gate at 8/8 so real matmuls start at 2.4 GHz
  - qkv projection as q^T,k^T (head-dim on partitions) and v (natural)
  - q/k head dims are host-interleaved in pairs (d, d+32) -> (2d, 2d+1),
    so RoPE's rotate_half is a within-quadrant partition pair-swap:
    one DVE stream_shuffle + elementwise combine against host-precomputed
    cos / sign-folded-sin tables.  No PE rotate matmul.
  - S^T = k'.q'^T per head pair, both heads' K=64 matmuls row-tiled into
    one PE pass (concurrent via tile_position row groups); exp on ScalarE
    with the softmax scale folded into the activation's free affine
  - PV as out^T = [v|1]^T @ E^T -- the ones column yields the softmax
    denominators in psum row 64; normalization deferred past PV:
    reciprocal on DVE, partition-broadcast via a DRAM bounce for interior
    pairs (latency hidden) and via indicator-weight matmuls into PSUM for
    the final pair (keeps the critical tail on-chip)
  - projection / out-projection matmul chunks are interleaved into the
    attention kc loop so the PE has filler work while each kc's exp cooks
  - out-projection's qc=0 half runs during the qc=1 attention loop; only
    the last few matmuls trail the end.
"""
import sys

sys.path.insert(0, "/opt/trn_rl_repo")

from collections import deque

import numpy as np
import ml_dtypes

import concourse.bass as bass
import concourse.tile as tile_mod
from concourse import mybir
from concourse.alu_op_type import AluOpType
from concourse.bass_utils import run_bass_kernel_spmd
from concourse.vector_clock import ScopedClock

F32 = mybir.dt.float32
BF16 = mybir.dt.bfloat16

B, N, D = 8, 1024, 768
H, DH = 12, 64
HP = H // 2          # head pairs (two heads share a 128-partition tile)
KC = D // 128        # contraction chunks for the projections
RC = N // 128        # row chunks of the sequence
NK = N // 128        # key chunks
SCALE = DH ** -0.5
N_WARM = 12          # HAM warmup matmuls during the input DMA


# --- walrus workaround: one sync-wait per instruction ---------------------
def _patched_drain_and_barrier(self, tick_clock, wait_clock):
    drain_inst = self.nc.sync.drain()
    wait_clock.add_sem_waits(
        drain_inst.ins, ScopedClock({None: tick_clock.global_clock})
    )
    si = drain_inst.ins.sync_info
    waits = list(si.on_wait or []) if si is not None else []
    if len(waits) > 1:
        drain_inst.ins.sync_info = mybir.SyncInfo(
            on_wait=waits[:1], on_update=list(si.on_update or [])
        )
        for w in waits[1:]:
            nop = self.nc.sync.nop(nofuse=True)
            nop.ins.sync_info = mybir.SyncInfo(on_wait=[w], on_update=[])
    self.nc.all_engine_barrier()
    assert self.sems is not None
    popped = self.nc._tile_sem_poison_stack.pop()
    assert popped is self._sem_poison
    self.nc.clear_and_free_semaphores(list(self.sems.allocated().values()))
    self.nc.all_engine_barrier()


tile_mod.TileContext._drain_and_barrier = _patched_drain_and_barrier


_split_counter = [0]


def split_sync_waits(nc, max_waits=1):
    """walrus rejects instructions carrying several sem waits; spill the
    excess onto engine-matched NOPs inserted directly before the offender."""
    for f in nc.m.functions:
        for bb in f.blocks:
            il = bb.instructions
            i = 0
            while i < len(il):
                inst = il[i]
                si = inst.sync_info
                waits = list(si.on_wait or []) if si is not None else []
                if len(waits) > max_waits:
                    inst.sync_info = mybir.SyncInfo(
                        on_wait=waits[:max_waits],
                        on_update=list(si.on_update or []),
                    )
                    rest = waits[max_waits:]
                    nops = []
                    for j in range(0, len(rest), max_waits):
                        _split_counter[0] += 1
                        nop = mybir.InstNoOp(
                            name=f"I-waitsplit-{_split_counter[0]}",
                            ins=[],
                            outs=[],
                            engine=inst.engine,
                        )
                        nop.sync_info = mybir.SyncInfo(
                            on_wait=rest[j : j + max_waits], on_update=[]
                        )
                        nops.append(nop)
                    for k, nop in enumerate(nops):
                        il.insert(i + k, nop)
                    i += len(nops)
                i += 1


def _bcast_rows(dram_ap, offset_elems, parts, free):
    """AP reading dram_ap[offset : offset+free] into `parts` partitions."""
    return bass.AP(
        tensor=dram_ap.tensor,
        offset=dram_ap.offset + offset_elems,
        ap=[[0, parts], [1, free]],
    )


def build_nc(with_bias=True):
    nc = bass.Bass()
    xt_d = nc.dram_tensor("xt", [D, N], BF16, kind="ExternalInput")
    wq_d = nc.dram_tensor("wq", [D, 3 * D], BF16, kind="ExternalInput")
    wo_d = nc.dram_tensor("wo", [D, D], BF16, kind="ExternalInput")
    bo_d = nc.dram_tensor("bo", [D], BF16, kind="ExternalInput")
    cos_d = nc.dram_tensor("cos2", [128, N], BF16, kind="ExternalInput")
    sin_d = nc.dram_tensor("sin2", [128, N], BF16, kind="ExternalInput")
    sel_d = nc.dram_tensor("sel", [8, 512], BF16, kind="ExternalInput")
    out_d = nc.dram_tensor("out", [D, N], BF16, kind="ExternalOutput")

    Exp = mybir.ActivationFunctionType.Exp
    Copy = mybir.ActivationFunctionType.Copy
    SWAP_MASK = [i ^ 1 for i in range(32)]

    with tile_mod.TileContext(nc) as tc:
        with (
            tc.tile_pool(name="singles", bufs=1) as singles,
            tc.tile_pool(name="work", bufs=2) as work,
            tc.tile_pool(name="dpool", bufs=1, space="DRAM") as dpool,
        ):
            recip_d = dpool.tile([2 * H * 512], BF16)
            recip_ap = recip_d[:]

            warm_w = singles.tile([128, 128], BF16)
            nc.vector.memset(warm_w[:], 0.0)
            warm_x = singles.tile([128, 512], BF16)
            nc.vector.memset(warm_x[:], 0.0)

            # input loads: xt on SyncE's DMA rings, wv (and the rest) on
            # ScalarE's (2nd HWDGE) so the two biggest input tensors
            # transfer on disjoint ring sets in parallel
            xt_sb = singles.tile([128, KC, N], BF16)
            wv_sb = singles.tile([128, KC, D], BF16)
            for kc in range(KC):
                nc.sync.dma_start(
                    out=xt_sb[:, kc, :], in_=xt_d[kc * 128 : (kc + 1) * 128, :]
                )
                nc.scalar.dma_start(
                    out=wv_sb[:, kc, :],
                    in_=wq_d[kc * 128 : (kc + 1) * 128, 2 * D : 3 * D],
                )
            # indicator weights for the final-pair reciprocal broadcast
            sel_sb = singles.tile([8, 4, 128], BF16)
            nc.scalar.dma_start(
                out=sel_sb[:], in_=sel_d[:].rearrange("p (j f) -> p j f", j=4)
            )
            cos_sb = singles.tile([128, N], BF16)
            nc.scalar.dma_start(out=cos_sb[:], in_=cos_d[:])
            sin_sb = singles.tile([128, N], BF16)
            nc.scalar.dma_start(out=sin_sb[:], in_=sin_d[:])
            bo_sb = singles.tile([1, D], BF16)
            nc.scalar.dma_start(
                out=bo_sb[:], in_=bo_d[:].rearrange("(o d) -> o d", o=1)
            )
            ones_sb = singles.tile([1, 512], BF16)
            nc.vector.memset(ones_sb[:], 1.0)
            wo_sb = singles.tile([128, KC, D], BF16)
            for c in range(KC):
                nc.scalar.dma_start(
                    out=wo_sb[:, c, :], in_=wo_d[c * 128 : (c + 1) * 128, :]
                )

            v_sb = singles.tile([128, NK, H, DH + 1], BF16)
            nc.gpsimd.memset(v_sb[:, :, :, DH : DH + 1], 1.0)
            junk_sb = singles.tile([1, 16], BF16)

            q_sb = singles.tile([128, KC, N], BF16)
            k_sb = singles.tile([128, KC, N], BF16)
            attnU_sb = singles.tile([128, KC, N], BF16)
            attn_sb = singles.tile([128, KC, N], BF16)

            # ---- v projection: v[rows, 768] = x @ Wv -------------------
            with tc.tile_pool(name="ps_v", bufs=2, space="PSUM") as ps_v:
                for i in range(N_WARM):
                    wp = ps_v.tile([128, D], F32, tag="v", name=f"warm{i}")
                    nc.tensor.matmul(
                        wp[:, 0:512], warm_w[:], warm_x[:], start=True, stop=True
                    )
                # preload the exp table set during the (ScalarE-idle)
                # v-projection phase instead of at the first real exp
                nc.scalar.activation(
                    out=junk_sb[:], in_=warm_x[0:1, 0:16], func=Exp, scale=1.0
                )
                for rc in range(RC):
                    vp = ps_v.tile([128, D], F32, tag="v")
                    for c0, w in ((0, 512), (512, 256)):
                        for kc in range(KC):
                            nc.tensor.matmul(
                                vp[:, c0 : c0 + w],
                                xt_sb[:, kc, rc * 128 : (rc + 1) * 128],
                                wv_sb[:, kc, c0 : c0 + w],
                                start=(kc == 0),
                                stop=(kc == KC - 1),
                            )
                    # strided copy into the [v | ones] per-head layout
                    nc.scalar.activation(
                        out=v_sb[:, rc, :, 0:DH],
                        in_=vp[:].rearrange("p (h d) -> p h d", h=H),
                        func=Copy,
                        scale=1.0,
                    )

            # ---- q^T / k^T projection + RoPE, split into filler chunks -
            # PE does only the 12 qkp matmuls; cast / pair-swap / rope
            # combine all run downstream on DVE.
            def proj_chunks(ps_proj, oc):
                col0 = oc * 128 if oc < KC else D + (oc - KC) * 128
                wt = work.tile(
                    [128, KC, 128], BF16, tag="wq", bufs=4, name=f"wt{oc}"
                )
                nc.sync.dma_start(
                    out=wt[:],
                    in_=wq_d[:, col0 : col0 + 128].rearrange(
                        "(c p) m -> p c m", c=KC
                    ),
                )
                q0 = work.tile([128, N], BF16, tag="q0", bufs=2, name=f"q0_{oc}")

                def half(qc2):
                    qkp = ps_proj.tile(
                        [128, 512], F32, tag="pq", name=f"qkp{oc}_{qc2}"
                    )
                    for kc in range(KC):
                        nc.tensor.matmul(
                            qkp[:],
                            wt[:, kc, :],
                            xt_sb[:, kc, qc2 * 512 : (qc2 + 1) * 512],
                            start=(kc == 0),
                            stop=(kc == KC - 1),
                        )
                    nc.vector.tensor_copy(
                        q0[:, qc2 * 512 : (qc2 + 1) * 512], qkp[:]
                    )

                def rope_tail():
                    q0s = work.tile(
                        [128, N], BF16, tag="q0s", bufs=2, name=f"q0s_{oc}"
                    )
                    nc.vector.stream_shuffle(q0s[:], q0[:], SWAP_MASK)
                    t1 = work.tile([128, N], BF16, tag="t1", bufs=2, name=f"t1_{oc}")
                    nc.vector.tensor_mul(t1[:], q0s[:], sin_sb[:])
                    t2 = work.tile([128, N], BF16, tag="t2", bufs=2, name=f"t2_{oc}")
                    nc.vector.tensor_mul(t2[:], q0[:], cos_sb[:])
                    dst = q_sb if oc < KC else k_sb
                    nc.vector.tensor_add(dst[:, oc % KC, :], t1[:], t2[:])

                return [lambda: half(0), lambda: half(1), rope_tail]

            # ---- attention: head pairs, row-tiled K=64 S^T matmuls -----
            def attn_pair(ps_att, qc, hp, filler, last=False):
                pvs = []
                for a in range(2):
                    pv = ps_att.tile(
                        [65, 512], F32, tag=f"pv{a}", bufs=1, name=f"pv{a}_{qc}_{hp}"
                    )
                    pvs.append(pv)
                for kc in range(NK):
                    st = ps_att.tile(
                        [128, N], F32, tag="st", bufs=2, name=f"st_{qc}_{hp}_{kc}"
                    )
                    for a in range(2):
                        po = 64 * a
                        nc.tensor.matmul(
                            st[:, a * 512 : (a + 1) * 512],
                            k_sb[po : po + 64, hp, kc * 128 : (kc + 1) * 128],
                            q_sb[po : po + 64, hp, qc * 512 : (qc + 1) * 512],
                            start=True,
                            stop=True,
                        )
                    e = work.tile([128, N], BF16, tag="e", bufs=2, name=f"e_{qc}_{hp}_{kc}")
                    nc.scalar.activation(out=e[:], in_=st[:], func=Exp, scale=SCALE)
                    # filler between exp and pv: PE streams useful work
                    # while this kc's exp cooks on ScalarE
                    if filler:
                        filler.popleft()()
                    for a in range(2):
                        nc.tensor.matmul(
                            pvs[a][:],
                            v_sb[:, kc, 2 * hp + a, :],
                            e[:, a * 512 : (a + 1) * 512],
                            start=(kc == 0),
                            stop=(kc == NK - 1),
                        )
                pvts = []
                for a in range(2):
                    pvt = work.tile(
                        [65, 512], BF16, tag="pvt", bufs=3, name=f"pvt{qc}_{2*hp+a}"
                    )
                    # final pair: split the two evacuations across DVE and
                    # the (now idle) ScalarE so they run in parallel
                    if last and a == 1:
                        nc.scalar.activation(
                            out=pvt[:], in_=pvs[a][:], func=Copy, scale=1.0
                        )
                    else:
                        nc.vector.tensor_copy(pvt[:], pvs[a][:])
                    nc.sync.dma_start(
                        out=attnU_sb[
                            64 * a : 64 * a + 64, hp, qc * 512 : (qc + 1) * 512
                        ],
                        in_=pvt[0:64, :],
                    )
                    pvts.append(pvt)
                sums_t = work.tile([8, 128], BF16, tag="sums", bufs=2, name=f"sums{qc}_{hp}")
                for a in range(2):
                    nc.sync.dma_start(
                        out=sums_t[a * 4 : a * 4 + 4, :], in_=pvts[a][64:65, :]
                    )
                return sums_t

            # ---- softmax normalization for one (head pair, qc) --------
            # interior pairs broadcast the reciprocal rows via a DRAM
            # bounce (latency hidden under later work); the final pair
            # broadcasts the raw denominator rows via K=1 matmuls into
            # PSUM and divides -- no reciprocal, no DMA on the tail
            def norm(hp, qc, sums_t, ps_last=None):
                rcp = work.tile([8, 128], BF16, tag="rcp", bufs=2, name=f"rcp{hp}_{qc}")
                with nc.allow_low_precision(
                    reason="bf16 softmax denominators; rel-err budget is 2e-2"
                ):
                    nc.vector.reciprocal(rcp[:], sums_t[:])
                if ps_last is not None:
                    rbp = ps_last.tile([128, 512], F32, tag="pq", name=f"rbp{hp}_{qc}")
                    for j in range(4):
                        nc.tensor.matmul(
                            rbp[:, j * 128 : (j + 1) * 128],
                            sel_sb[:, j, :],
                            rcp[:],
                            start=(j == 0),
                            stop=(j == 3),
                        )
                    nc.vector.tensor_mul(
                        attn_sb[:, hp, qc * 512 : (qc + 1) * 512],
                        attnU_sb[:, hp, qc * 512 : (qc + 1) * 512],
                        rbp[:],
                    )
                    return
                o0 = (qc * H + 2 * hp) * 512
                nc.sync.dma_start(
                    out=recip_d[o0 : o0 + 1024].rearrange("(p f) -> p f", p=8),
                    in_=rcp[:],
                )
                rb = work.tile([128, 512], BF16, tag="rb", bufs=3, name=f"rb{hp}_{qc}")
                nc.sync.dma_start(
                    out=rb[0:64, :], in_=_bcast_rows(recip_ap, o0, 64, 512)
                )
                nc.sync.dma_start(
                    out=rb[64:128, :], in_=_bcast_rows(recip_ap, o0 + 512, 64, 512)
                )
                nc.vector.tensor_mul(
                    attn_sb[:, hp, qc * 512 : (qc + 1) * 512],
                    attnU_sb[:, hp, qc * 512 : (qc + 1) * 512],
                    rb[:],
                )

            # ---- out-projection for one (128-col tile, qc) -------------
            def outproj_chunks(ps_fin, oc, qc, tail_cast_act=False):
                fp = ps_fin.tile([128, 512], F32, tag="pq", name=f"fin{oc}_{qc}")

                def cpair(c0):
                    last = c0 + 2 >= KC
                    for c in (c0, c0 + 1):
                        nc.tensor.matmul(
                            fp[:],
                            wo_sb[:, c, oc * 128 : (oc + 1) * 128],
                            attn_sb[:, c, qc * 512 : (qc + 1) * 512],
                            start=(c == 0),
                            stop=(not with_bias and c == KC - 1),
                        )
                    if not last:
                        return
                    if with_bias:
                        nc.tensor.matmul(
                            fp[:],
                            bo_sb[0:1, oc * 128 : (oc + 1) * 128],
                            ones_sb[:],
                            start=False,
                            stop=True,
                        )
                    fsb = work.tile(
                        [128, 512], BF16, tag="fsb", bufs=3, name=f"fsb{oc}_{qc}"
                    )
                    if tail_cast_act:
                        nc.scalar.activation(
                            out=fsb[:], in_=fp[:], func=Copy, scale=1.0
                        )
                    else:
                        nc.vector.tensor_copy(fsb[:], fp[:])
                    eng = nc.scalar if tail_cast_act else nc.sync
                    eng.dma_start(
                        out=out_d[
                            oc * 128 : (oc + 1) * 128, qc * 512 : (qc + 1) * 512
                        ],
                        in_=fsb[:],
                    )

                return [lambda: cpair(0), lambda: cpair(2), lambda: cpair(4)]

            with (
                tc.tile_pool(name="ps_att", bufs=1, space="PSUM") as ps_att,
                tc.tile_pool(name="ps_x", bufs=2, space="PSUM") as ps_x,
            ):
                pend = deque()
                for f in proj_chunks(ps_x, 0) + proj_chunks(ps_x, KC):
                    f()
                for hp in range(HP):
                    if hp + 1 < HP:
                        pend.extend(proj_chunks(ps_x, hp + 1))
                        pend.extend(proj_chunks(ps_x, KC + hp + 1))
                    s_t = attn_pair(ps_att, 0, hp, pend)
                    while pend:
                        pend.popleft()()
                    norm(hp, 0, s_t)
                for hp in range(HP):
                    pend.extend(outproj_chunks(ps_x, hp, 0))
                    last = hp == HP - 1
                    s_t = attn_pair(ps_att, 1, hp, pend, last=last)
                    while pend:
                        pend.popleft()()
                    norm(hp, 1, s_t, ps_last=ps_x if last else None)
                for oc in range(KC):
                    for f in outproj_chunks(ps_x, oc, 1, tail_cast_act=(oc % 2 == 1)):
                        f()

    split_sync_waits(nc, max_waits=1)
    return nc


def _host_prep(x, w_qkv, w_out, b_out):
    bf = ml_dtypes.bfloat16
    inv_freq = 1.0 / (10000.0 ** (np.arange(0, DH, 2, dtype=np.float32) / DH))
    t = np.arange(N, dtype=np.float32)
    freqs = np.outer(t, inv_freq)
    emb = np.concatenate([freqs, freqs], axis=1)        # [N, DH]
    cos_t = np.cos(emb).T.astype(np.float32)            # [DH, N]
    sin_t = np.sin(emb).T.astype(np.float32)

    # interleave head dims in pairs (d, d+32) -> rows (2d, 2d+1) so
    # rotate_half becomes a partition pair-swap; fold rotate's sign into
    # the sin table (row 2d carries -sin)
    perm64 = np.ravel(
        np.stack([np.arange(32), np.arange(32) + 32], axis=1)
    )                                                   # [0,32,1,33,...]
    signs = np.tile(np.array([-1.0, 1.0], np.float32), 32)[:, None]
    cos_p = cos_t[perm64]
    sin_p = sin_t[perm64] * signs
    cos2 = np.tile(cos_p, (2, 1)).astype(bf)
    sin2 = np.tile(sin_p, (2, 1)).astype(bf)

    inner = H * DH
    qk_perm = np.concatenate([h * DH + perm64 for h in range(H)])
    wq = np.asarray(w_qkv, dtype=np.float32).copy()
    wq[:, 0:inner] = wq[:, 0:inner][:, qk_perm]
    wq[:, inner : 2 * inner] = wq[:, inner : 2 * inner][:, qk_perm]

    # indicator weights for the final-pair reciprocal broadcast:
    # sel[h*4+j, j, h*64:(h+1)*64] = 1
    sel = np.zeros((8, 4, 128), np.float32)
    for h in range(2):
        for j in range(4):
            sel[h * 4 + j, j, h * 64 : (h + 1) * 64] = 1.0

    xt = np.ascontiguousarray(x.transpose(0, 2, 1)).astype(bf)
    shared = {
        "wq": np.ascontiguousarray(wq).astype(bf),
        "wo": np.ascontiguousarray(w_out).astype(bf),
        "bo": np.ascontiguousarray(b_out).astype(bf),
        "cos2": np.ascontiguousarray(cos2),
        "sin2": np.ascontiguousarray(sin2),
        "sel": np.ascontiguousarray(sel.reshape(8, 512)).astype(bf),
    }
    return [dict(shared, xt=np.ascontiguousarray(xt[i])) for i in range(B)]


_NC_CACHE = {}
LAST_EXEC_NS = [None]


def _run(in_maps, trace=False, with_bias=True):
    if with_bias not in _NC_CACHE:
        _NC_CACHE[with_bias] = build_nc(with_bias=with_bias)
    res = run_bass_kernel_spmd(
        _NC_CACHE[with_bias], in_maps, list(range(B)), trace=trace
    )
    LAST_EXEC_NS[0] = res.exec_time_ns
    out_t = np.stack(
        [np.asarray(res.results[i]["out"]).astype(np.float32) for i in range(B)]
    )
    return np.ascontiguousarray(out_t.transpose(0, 2, 1))


def kernel(x, w_qkv, w_out, b_out, _trace=False):
    b_out = np.asarray(b_out, dtype=np.float32)
    in_maps = _host_prep(
        np.asarray(x, dtype=np.float32),
        np.asarray(w_qkv, dtype=np.float32),
        np.asarray(w_out, dtype=np.float32),
        b_out,
    )
    return _run(in_maps, trace=_trace, with_bias=bool(np.any(b_out)))
